# revision 13
# baseline (speedup 1.0000x reference)
"""GAT-style attention head (gnn_message_passing) on 8 Trainium2 NeuronCores.

Math (per node b with N=32 neighbors, D=U=128):
    c_n = W @ a_neigh ; c_s = W @ a_self                  (tiny, host)
    nf1[b,n] = neigh[b,n,:] . c_n
    sf1[b]   = self[b,:] . c_s
    e        = exp(lrelu_0.2(sf1[b] + nf1[b,n]))
    agg[b,:] = sum_n e[b,n] * neigh[b,n,:]
    out[b,:] = lrelu_0.2((agg[b,:] @ W) / sum_n e[b,n])

Device-side formulation: the HOST pre-multiplies neigh by c_n (prod =
neigh * c_n, staged bf16) and pre-divides W rows by c_n (W' = W / c_n).
Then on device:
    nf1[b,n] = row-sum of prod[b,n,:]        (bf16 add-tree, GPSIMD+DVE)
    aggP     = sum_n e[b,n] * prod[b,n,:]    (PE, block-diag e4)
    out      = prelu((aggP @ W') * (1/Z))    (PE + one ACT Prelu(scale))
since agg @ W == (aggP / c_n) @ W == aggP @ W'.

Self path: host stages selfT[d, node] (transposed, bf16). sf1e lands
directly in the (g,q)-tile layout via 4 small PE matmuls per supertile:
  sf1e[32g:32g+32, t] = sum_d CS[d, q] * selfT[d, 4t+g]   (CS cols = c_s)
so no DRAM-roundtrip shuffle and no partition broadcast are needed.

Sharding: batch across 8 cores (6250 nodes/core, padded to 6272 = 49
supertiles of 128 nodes). A supertile is 32 tiles of 128 (b,n)-rows;
tile t holds nodes 4t..4t+3 (partition p = 32*g + q -> node 4t+g,
neighbor q). Host stages each supertile partition-contiguously so the
per-supertile load is ONE ~1 MiB DMA with 8 KiB contiguous runs; it is
the only DMA on the sync ring (the y store goes out on the scalar ring)
so loads prefetch deeply without head-of-line blocking.
"""

import numpy as np
from ml_dtypes import bfloat16

B, N, D = 50000, 32, 128
NCORES = 8
NODES_PER_CORE = B // NCORES            # 6250
SUPER = 128                              # nodes per supertile
NS = (NODES_PER_CORE + SUPER - 1) // SUPER   # 49 supertiles
NODES_PAD = NS * SUPER                   # 6272
NTILES = 32                              # tiles (of 128 rows) per supertile
ALPHA = 0.2
K1 = 20                                  # tree level-1 tiles done on GPSIMD


def build_program(ns=NS):
    from concourse import mybir
    from concourse.bacc import Bacc
    from concourse.bass import ds
    from concourse.tile import TileContext

    f32 = mybir.dt.float32
    bf16 = mybir.dt.bfloat16
    nc = Bacc()
    nodes_pad = ns * SUPER

    xin = nc.declare_dram_parameter("xin", [nodes_pad, NTILES * D], bf16, isOutput=False)
    selft = nc.declare_dram_parameter("selft", [128, nodes_pad], bf16, isOutput=False)
    w_in = nc.declare_dram_parameter("wmat", [D, D], bf16, isOutput=False)
    cs_in = nc.declare_dram_parameter("csrep", [128, 32], bf16, isOutput=False)
    ones1_in = nc.declare_dram_parameter("ones1", [128, 1], bf16, isOutput=False)
    out_d = nc.declare_dram_parameter("out", [nodes_pad, D], f32, isOutput=True)

    add = mybir.AluOpType.add
    AF = mybir.ActivationFunctionType
    AX = mybir.AxisListType

    with TileContext(nc) as tc:
        with (
            tc.tile_pool(name="consts", bufs=1) as cpool,
            tc.tile_pool(name="x", bufs=12) as xpool,
            tc.tile_pool(name="t1p", bufs=4) as t1pool,
            tc.tile_pool(name="t2p", bufs=4) as t2pool,
            tc.tile_pool(name="work", bufs=4) as wpool,
            tc.tile_pool(name="small", bufs=6) as spool,
            tc.tile_pool(name="e4p", bufs=1) as e4pool,
            tc.tile_pool(name="psA", bufs=2, space="PSUM") as psA,
            tc.tile_pool(name="psB", bufs=2, space="PSUM") as psB,
            tc.tile_pool(name="psZ", bufs=2, space="PSUM") as psZ,
            tc.tile_pool(name="psS", bufs=3, space="PSUM") as psS,
        ):
            w_sb = cpool.tile([128, D], bf16, tag="w")
            cs_sb = cpool.tile([128, 32], bf16, tag="cs")
            ones1 = cpool.tile([128, 1], bf16, tag="ones1")
            selft_sb = cpool.tile([128, nodes_pad], bf16, tag="selft")
            nc.sync.dma_start(out=w_sb, in_=w_in[:, :])
            nc.sync.dma_start(out=cs_sb, in_=cs_in[:, :])
            nc.sync.dma_start(out=ones1, in_=ones1_in[:, :])
            nc.sync.dma_start(out=selft_sb, in_=selft[:, :])

            # Three stable e4 buffers: block-diagonal scatter of e values. The
            # off-block regions are zeroed ONCE and never written again.
            e4_bufs = []
            for q in range(4):
                t = e4pool.tile([128, NTILES * 4], bf16, tag=f"e4_{q}")
                nc.vector.memset(t, 0.0)
                e4_bufs.append(t)

            # Software-pipelined loop: phase A(s) = load + logits + e4;
            # phase B(s) = aggregation + normalize + project + store, emitted
            # one iteration later so no engine queue ever waits on the tail
            # of the in-flight supertile.
            state = {}
            sf1e_state = {}

            def emit_sf1e(s):
                # sf1e via 4 small PE matmuls (see module docstring), emitted
                # one supertile early so the F1 add never waits on the PE.
                sf1e_ps = psS.tile([128, NTILES], f32, tag="sf1e")
                sview = selft_sb[:, ds(SUPER * s, SUPER)].rearrange(
                    "p (t g) -> p g t", g=4
                )
                for g in range(4):
                    nc.tensor.matmul(
                        sf1e_ps[ds(32 * g, 32), :],
                        cs_sb,
                        sview[:, ds(g, 1), :].squeeze(1),
                        start=True, stop=True,
                        tile_position=(0, 32 * g),
                    )
                sf1e_state[s] = sf1e_ps

            def phase_a(s):
                x = xpool.tile([128, NTILES * D], bf16, tag="x")
                # alternate the two HWDGE rings (SP / ACT) so the x stream
                # drains through two DMA queues concurrently
                xeng = nc.sync if s % 2 == 0 else nc.scalar
                xeng.dma_start(out=x, in_=xin[ds(SUPER * s, SUPER), :])
                if s + 1 < ns:
                    emit_sf1e(s + 1)

                # nf1 = row-sums of prod, via bf16 add-tree (all DVE 2x)
                xv = x.rearrange("p (t h d) -> p t h d", h=2, d=64)
                t1 = t1pool.tile([128, NTILES * 64], bf16, tag="t1")
                t1v = t1.rearrange("p (t d) -> p t d", d=64)
                nc.vector.tensor_tensor(
                    t1v,
                    xv[:, :, ds(0, 1), :].squeeze(2),
                    xv[:, :, ds(1, 1), :].squeeze(2),
                    op=add,
                )
                t1h = t1.rearrange("p (t h d) -> p t h d", h=2, d=32)
                t2 = t2pool.tile([128, NTILES * 32], bf16, tag="t2")
                t2v = t2.rearrange("p (t d) -> p t d", d=32)
                nc.vector.tensor_tensor(
                    t2v,
                    t1h[:, :, ds(0, 1), :].squeeze(2),
                    t1h[:, :, ds(1, 1), :].squeeze(2),
                    op=add,
                )
                t2h = t2.rearrange("p (t h d) -> p t h d", h=2, d=16)
                t3 = t2pool.tile([128, NTILES * 16], bf16, tag="t3")
                t3v = t3.rearrange("p (t d) -> p t d", d=16)
                nc.vector.tensor_tensor(
                    t3v,
                    t2h[:, :, ds(0, 1), :].squeeze(2),
                    t2h[:, :, ds(1, 1), :].squeeze(2),
                    op=add,
                )
                F1 = spool.tile([128, NTILES], f32, tag="F1")
                nc.vector.tensor_reduce(F1, t3v, AX.X, add)

                # logits -> prelu; exp writes straight into e4 blocks
                nc.vector.tensor_add(F1, F1, sf1e_state.pop(s))
                lk = spool.tile([128, NTILES], f32, tag="lk")
                nc.scalar.activation(lk, F1, AF.Prelu, alpha=ALPHA)
                e4 = e4_bufs[s % 4]
                e4v = e4.rearrange("p (t g) -> p t g", g=4)
                for g in range(4):
                    nc.scalar.activation(
                        e4v[ds(32 * g, 32), :, ds(g, 1)],
                        lk[ds(32 * g, 32), :].unsqueeze(2),
                        AF.Exp,
                    )
                state[s] = (x, e4)

            def phase_b(s):
                x, e4 = state.pop(s)
                # Z first (so the DVE reciprocal never waits on the agg
                # burst); it rides the same PSUM tile in column 128
                aggT_ps = psA.tile([128, 132], f32, tag="aggT")
                nc.tensor.matmul(
                    aggT_ps[:, ds(128, 1)], e4, ones1, start=True, stop=True
                )
                zinv = spool.tile([128, 1], f32, tag="zinv")
                nc.vector.reciprocal(zinv, aggT_ps[:, ds(128, 1)])
                for i in range(NTILES):
                    nc.tensor.matmul(
                        aggT_ps[:, ds(4 * i, 4)],
                        x[:, ds(D * i, D)],
                        e4[:, ds(4 * i, 4)],
                        start=True, stop=True,
                    )

                # final projection + fused scale+prelu
                aggT16 = wpool.tile([128, 128], bf16, tag="aggT16")
                nc.scalar.copy(aggT16, aggT_ps[:, ds(0, 128)])
                out_ps = psB.tile([128, 128], f32, tag="outps")
                nc.tensor.matmul(out_ps, aggT16, w_sb, start=True, stop=True)
                y = wpool.tile([128, 128], f32, tag="y")
                nc.scalar.activation(y, out_ps, AF.Prelu, scale=zinv[:, :], alpha=ALPHA)
                # y store rides the GPSIMD (SWDGE) path: the sync ring stays
                # free for x prefetches and the scalar queue for ACT work.
                nc.gpsimd.dma_start(out=out_d[ds(SUPER * s, SUPER), :], in_=y)

            emit_sf1e(0)
            phase_a(0)
            for s in range(1, ns):
                phase_a(s)
                phase_b(s - 1)
            phase_b(ns - 1)

    nc.compile()
    return nc


_CACHE = {}


def _get_program():
    if "nc" not in _CACHE:
        _CACHE["nc"] = build_program()
    return _CACHE["nc"]


def make_in_maps(self_vecs, neigh_vecs, W, a_self, a_neigh):
    """Host staging: fold c_n into neigh (bf16), W/c_n; transpose self."""
    self_vecs = np.ascontiguousarray(self_vecs, dtype=np.float32)
    neigh_vecs = np.ascontiguousarray(neigh_vecs, dtype=np.float32)
    W = np.ascontiguousarray(W, dtype=np.float32)
    c_n = (W @ np.asarray(a_neigh, dtype=np.float32)[:, 0]).astype(np.float32)
    c_s = (W @ np.asarray(a_self, dtype=np.float32)[:, 0]).astype(np.float32)
    wmat = (W / c_n[:, None]).astype(bfloat16)

    consts = {
        "wmat": wmat,
        "csrep": np.tile(c_s[:, None], (1, 32)).astype(bfloat16),
        "ones1": np.ones((128, 1), dtype=bfloat16),
    }

    in_maps = []
    for k in range(NCORES):
        lo = k * NODES_PER_CORE
        hi = lo + NODES_PER_CORE
        # neigh: premult by c_n, pad, supertile-partition layout
        P = np.zeros((NODES_PAD, N, D), dtype=bfloat16)
        P[:NODES_PER_CORE] = neigh_vecs[lo:hi] * c_n[None, None, :]
        # [s, t, g, q, d] -> rows s*128 + 32g + q, free (t, d)
        A = (
            P.reshape(NS, NTILES, 4, N, D)
            .transpose(0, 2, 3, 1, 4)
            .reshape(NODES_PAD, NTILES * D)
        )
        A = np.ascontiguousarray(A)
        # self: transposed [d, node], bf16
        SP = np.zeros((NODES_PAD, D), dtype=np.float32)
        SP[:NODES_PER_CORE] = self_vecs[lo:hi]
        ST = np.ascontiguousarray(SP.T).astype(bfloat16)
        in_maps.append({"xin": A, "selft": ST, **consts})
    return in_maps


def gather_out(results):
    out = np.empty((B, D), dtype=np.float32)
    for k in range(NCORES):
        out[k * NODES_PER_CORE : (k + 1) * NODES_PER_CORE] = results[k]["out"][
            :NODES_PER_CORE
        ]
    return out


def kernel(self_vecs, neigh_vecs, W, a_self, a_neigh):
    from concourse.bass_utils import run_bass_kernel_spmd

    in_maps = make_in_maps(self_vecs, neigh_vecs, W, a_self, a_neigh)
    nc = _get_program()
    res = run_bass_kernel_spmd(nc, in_maps, list(range(NCORES)))
    return gather_out(res.results)


# revision 14
# speedup vs baseline: 1.0675x; 1.0675x over previous
"""GAT-style attention head (gnn_message_passing) on 8 Trainium2 NeuronCores.

Math (per node b with N=32 neighbors, D=U=128):
    c_n = W @ a_neigh ; c_s = W @ a_self                  (tiny, host)
    nf1[b,n] = neigh[b,n,:] . c_n
    sf1[b]   = self[b,:] . c_s
    e        = exp(lrelu_0.2(sf1[b] + nf1[b,n]))
    agg[b,:] = sum_n e[b,n] * neigh[b,n,:]
    out[b,:] = lrelu_0.2((agg[b,:] @ W) / sum_n e[b,n])

Device-side formulation: the HOST pre-multiplies neigh by c_n (prod =
neigh * c_n, staged bf16) and pre-divides W rows by c_n (W' = W / c_n).
Then on device:
    nf1[b,n] = row-sum of prod[b,n,:]        (bf16 add-tree, GPSIMD+DVE)
    aggP     = sum_n e[b,n] * prod[b,n,:]    (PE, block-diag e4)
    out      = prelu((aggP @ W') * (1/Z))    (PE + one ACT Prelu(scale))
since agg @ W == (aggP / c_n) @ W == aggP @ W'.

Self path: host stages selfT[d, node] (transposed, bf16). sf1e lands
directly in the (g,q)-tile layout via 4 small PE matmuls per supertile:
  sf1e[32g:32g+32, t] = sum_d CS[d, q] * selfT[d, 4t+g]   (CS cols = c_s)
so no DRAM-roundtrip shuffle and no partition broadcast are needed.

Sharding: batch across 8 cores (6250 nodes/core, padded to 6272 = 49
supertiles of 128 nodes). A supertile is 32 tiles of 128 (b,n)-rows;
tile t holds nodes 4t..4t+3 (partition p = 32*g + q -> node 4t+g,
neighbor q). Host stages each supertile partition-contiguously so the
per-supertile load is ONE ~1 MiB DMA with 8 KiB contiguous runs; it is
the only DMA on the sync ring (the y store goes out on the scalar ring)
so loads prefetch deeply without head-of-line blocking.
"""

import numpy as np
from ml_dtypes import bfloat16

B, N, D = 50000, 32, 128
NCORES = 8
NODES_PER_CORE = B // NCORES            # 6250
SUPER = 128                              # nodes per supertile
NS = (NODES_PER_CORE + SUPER - 1) // SUPER   # 49 supertiles
NODES_PAD = NS * SUPER                   # 6272
NTILES = 32                              # tiles (of 128 rows) per supertile
ALPHA = 0.2
K1 = 20                                  # tree level-1 tiles done on GPSIMD


def build_program(ns=NS):
    from concourse import mybir
    from concourse.bacc import Bacc
    from concourse.bass import ds
    from concourse.tile import TileContext

    f32 = mybir.dt.float32
    bf16 = mybir.dt.bfloat16
    nc = Bacc()
    nodes_pad = ns * SUPER

    xin = nc.declare_dram_parameter("xin", [nodes_pad, NTILES * D], bf16, isOutput=False)
    selft = nc.declare_dram_parameter("selft", [128, nodes_pad], bf16, isOutput=False)
    w_in = nc.declare_dram_parameter("wmat", [D, D], bf16, isOutput=False)
    cs_in = nc.declare_dram_parameter("csrep", [128, 32], bf16, isOutput=False)
    ones1_in = nc.declare_dram_parameter("ones1", [128, 1], bf16, isOutput=False)
    out_d = nc.declare_dram_parameter("out", [nodes_pad, D], f32, isOutput=True)

    add = mybir.AluOpType.add
    AF = mybir.ActivationFunctionType
    AX = mybir.AxisListType

    with TileContext(nc) as tc:
        with (
            tc.tile_pool(name="consts", bufs=1) as cpool,
            tc.tile_pool(name="x", bufs=6) as xpool,
            tc.tile_pool(name="t1p", bufs=3) as t1pool,
            tc.tile_pool(name="t2p", bufs=3) as t2pool,
            tc.tile_pool(name="work", bufs=3) as wpool,
            tc.tile_pool(name="small", bufs=4) as spool,
            tc.tile_pool(name="e4p", bufs=1) as e4pool,
            tc.tile_pool(name="psA", bufs=2, space="PSUM") as psA,
            tc.tile_pool(name="psB", bufs=2, space="PSUM") as psB,
            tc.tile_pool(name="psZ", bufs=2, space="PSUM") as psZ,
            tc.tile_pool(name="psS", bufs=3, space="PSUM") as psS,
        ):
            w_sb = cpool.tile([128, D], bf16, tag="w")
            cs_sb = cpool.tile([128, 32], bf16, tag="cs")
            ones1 = cpool.tile([128, 1], bf16, tag="ones1")
            selft_sb = cpool.tile([128, nodes_pad], bf16, tag="selft")
            nc.sync.dma_start(out=w_sb, in_=w_in[:, :])
            nc.sync.dma_start(out=cs_sb, in_=cs_in[:, :])
            nc.sync.dma_start(out=ones1, in_=ones1_in[:, :])
            nc.gpsimd.dma_start(out=selft_sb, in_=selft[:, :])

            # Three stable e4 buffers: block-diagonal scatter of e values. The
            # off-block regions are zeroed ONCE and never written again.
            e4_bufs = []
            for q in range(3):
                t = e4pool.tile([128, NTILES * 4], bf16, tag=f"e4_{q}")
                nc.vector.memset(t, 0.0)
                e4_bufs.append(t)

            # Software-pipelined loop: phase A(s) = load + logits + e4;
            # phase B(s) = aggregation + normalize + project + store, emitted
            # one iteration later so no engine queue ever waits on the tail
            # of the in-flight supertile.
            state = {}
            sf1e_state = {}

            def emit_sf1e(s):
                # sf1e via 4 small PE matmuls (see module docstring), emitted
                # one supertile early so the F1 add never waits on the PE.
                sf1e_ps = psS.tile([128, NTILES], f32, tag="sf1e")
                sview = selft_sb[:, ds(SUPER * s, SUPER)].rearrange(
                    "p (t g) -> p g t", g=4
                )
                for g in range(4):
                    nc.tensor.matmul(
                        sf1e_ps[ds(32 * g, 32), :],
                        cs_sb,
                        sview[:, ds(g, 1), :].squeeze(1),
                        start=True, stop=True,
                        tile_position=(0, 32 * g),
                    )
                sf1e_state[s] = sf1e_ps

            def phase_a(s):
                x = xpool.tile([128, NTILES * D], bf16, tag="x")
                # two concurrent half-transfers on the two HWDGE rings (SP /
                # ACT) halve the per-supertile load latency
                half = NTILES * D // 2
                nc.sync.dma_start(
                    out=x[:, ds(0, half)], in_=xin[ds(SUPER * s, SUPER), ds(0, half)]
                )
                nc.scalar.dma_start(
                    out=x[:, ds(half, half)],
                    in_=xin[ds(SUPER * s, SUPER), ds(half, half)],
                )
                if s + 1 < ns:
                    emit_sf1e(s + 1)

                # nf1 = row-sums of prod, via bf16 add-tree (all DVE 2x)
                xv = x.rearrange("p (t h d) -> p t h d", h=2, d=64)
                t1 = t1pool.tile([128, NTILES * 64], bf16, tag="t1")
                t1v = t1.rearrange("p (t d) -> p t d", d=64)
                nc.vector.tensor_tensor(
                    t1v,
                    xv[:, :, ds(0, 1), :].squeeze(2),
                    xv[:, :, ds(1, 1), :].squeeze(2),
                    op=add,
                )
                t1h = t1.rearrange("p (t h d) -> p t h d", h=2, d=32)
                t2 = t2pool.tile([128, NTILES * 32], bf16, tag="t2")
                t2v = t2.rearrange("p (t d) -> p t d", d=32)
                nc.vector.tensor_tensor(
                    t2v,
                    t1h[:, :, ds(0, 1), :].squeeze(2),
                    t1h[:, :, ds(1, 1), :].squeeze(2),
                    op=add,
                )
                t2h = t2.rearrange("p (t h d) -> p t h d", h=2, d=16)
                t3 = t2pool.tile([128, NTILES * 16], bf16, tag="t3")
                t3v = t3.rearrange("p (t d) -> p t d", d=16)
                nc.vector.tensor_tensor(
                    t3v,
                    t2h[:, :, ds(0, 1), :].squeeze(2),
                    t2h[:, :, ds(1, 1), :].squeeze(2),
                    op=add,
                )
                F1 = spool.tile([128, NTILES], f32, tag="F1")
                nc.vector.tensor_reduce(F1, t3v, AX.X, add)

                # logits -> prelu; exp writes straight into e4 blocks
                nc.vector.tensor_add(F1, F1, sf1e_state.pop(s))
                lk = spool.tile([128, NTILES], f32, tag="lk")
                nc.scalar.activation(lk, F1, AF.Prelu, alpha=ALPHA)
                e4 = e4_bufs[s % 3]
                e4v = e4.rearrange("p (t g) -> p t g", g=4)
                for g in range(4):
                    nc.scalar.activation(
                        e4v[ds(32 * g, 32), :, ds(g, 1)],
                        lk[ds(32 * g, 32), :].unsqueeze(2),
                        AF.Exp,
                    )
                state[s] = (x, e4)

            def phase_b(s):
                x, e4 = state.pop(s)
                # Z first (so the DVE reciprocal never waits on the agg
                # burst); it rides the same PSUM tile in column 128
                aggT_ps = psA.tile([128, 132], f32, tag="aggT")
                nc.tensor.matmul(
                    aggT_ps[:, ds(128, 1)], e4, ones1, start=True, stop=True
                )
                zinv = spool.tile([128, 1], f32, tag="zinv")
                nc.vector.reciprocal(zinv, aggT_ps[:, ds(128, 1)])
                for i in range(NTILES):
                    nc.tensor.matmul(
                        aggT_ps[:, ds(4 * i, 4)],
                        x[:, ds(D * i, D)],
                        e4[:, ds(4 * i, 4)],
                        start=True, stop=True,
                    )

                # final projection + fused scale+prelu
                aggT16 = wpool.tile([128, 128], bf16, tag="aggT16")
                nc.scalar.copy(aggT16, aggT_ps[:, ds(0, 128)])
                out_ps = psB.tile([128, 128], f32, tag="outps")
                nc.tensor.matmul(out_ps, aggT16, w_sb, start=True, stop=True)
                y = wpool.tile([128, 128], f32, tag="y")
                nc.scalar.activation(y, out_ps, AF.Prelu, scale=zinv[:, :], alpha=ALPHA)
                # y store rides the GPSIMD (SWDGE) path: the sync ring stays
                # free for x prefetches and the scalar queue for ACT work.
                nc.gpsimd.dma_start(out=out_d[ds(SUPER * s, SUPER), :], in_=y)

            emit_sf1e(0)
            phase_a(0)
            for s in range(1, ns):
                phase_a(s)
                phase_b(s - 1)
            phase_b(ns - 1)

    nc.compile()
    return nc


_CACHE = {}


def _get_program():
    if "nc" not in _CACHE:
        _CACHE["nc"] = build_program()
    return _CACHE["nc"]


def make_in_maps(self_vecs, neigh_vecs, W, a_self, a_neigh):
    """Host staging: fold c_n into neigh (bf16), W/c_n; transpose self."""
    self_vecs = np.ascontiguousarray(self_vecs, dtype=np.float32)
    neigh_vecs = np.ascontiguousarray(neigh_vecs, dtype=np.float32)
    W = np.ascontiguousarray(W, dtype=np.float32)
    c_n = (W @ np.asarray(a_neigh, dtype=np.float32)[:, 0]).astype(np.float32)
    c_s = (W @ np.asarray(a_self, dtype=np.float32)[:, 0]).astype(np.float32)
    wmat = (W / c_n[:, None]).astype(bfloat16)

    consts = {
        "wmat": wmat,
        "csrep": np.tile(c_s[:, None], (1, 32)).astype(bfloat16),
        "ones1": np.ones((128, 1), dtype=bfloat16),
    }

    in_maps = []
    for k in range(NCORES):
        lo = k * NODES_PER_CORE
        hi = lo + NODES_PER_CORE
        # neigh: premult by c_n, pad, supertile-partition layout
        P = np.zeros((NODES_PAD, N, D), dtype=bfloat16)
        P[:NODES_PER_CORE] = neigh_vecs[lo:hi] * c_n[None, None, :]
        # [s, t, g, q, d] -> rows s*128 + 32g + q, free (t, d)
        A = (
            P.reshape(NS, NTILES, 4, N, D)
            .transpose(0, 2, 3, 1, 4)
            .reshape(NODES_PAD, NTILES * D)
        )
        A = np.ascontiguousarray(A)
        # self: transposed [d, node], bf16
        SP = np.zeros((NODES_PAD, D), dtype=np.float32)
        SP[:NODES_PER_CORE] = self_vecs[lo:hi]
        ST = np.ascontiguousarray(SP.T).astype(bfloat16)
        in_maps.append({"xin": A, "selft": ST, **consts})
    return in_maps


def gather_out(results):
    out = np.empty((B, D), dtype=np.float32)
    for k in range(NCORES):
        out[k * NODES_PER_CORE : (k + 1) * NODES_PER_CORE] = results[k]["out"][
            :NODES_PER_CORE
        ]
    return out


def kernel(self_vecs, neigh_vecs, W, a_self, a_neigh):
    from concourse.bass_utils import run_bass_kernel_spmd

    in_maps = make_in_maps(self_vecs, neigh_vecs, W, a_self, a_neigh)
    nc = _get_program()
    res = run_bass_kernel_spmd(nc, in_maps, list(range(NCORES)))
    return gather_out(res.results)
